# revision 38
# baseline (speedup 1.0000x reference)
"""Additive (Bahdanau) attention on 8 TRN2 NeuronCores.

Problem: B=8, LQ=256, LK=1024, DQ=DK=DV=512, H=128.
  q = Q @ W_q; k = K @ W_k
  scores[b,q,k] = sum_h w_v[h] * tanh(qf[b,q,h] + kf[b,k,h])
  out = softmax_k(mask(scores)) @ V

Factorized-score formulation (replaces the O(LQ*LK*H) elementwise tanh):
  tanh(u+v) ~= sum_{m=0..9} Cq_m(u) * t(v)^m,   t = clamp(v,+-3.4)/1.9
with Cq_m(u) = sum_i beta[i,m] T_i(clamp(u,+-3.4)/3.4) a 40-atom pruned
Chebyshev fit (offline ridge LS on the empirical qf/kf distribution;
end-to-end rel err ~6.4e-3 including all f16 effects). Scores become 10
accumulating PE matmuls per 128-key chunk:
  scores[k,q] = sum_m matmul(lhsT = t^m [h,k], rhs = (w_v o Cq_m) [h,q])

Sharding: batches are paired (largest with smallest valid length) onto
core pairs. Each core loads ONLY its pair's K/V (about 2.6MB instead of
the full 9.4MB - no 8x DMA replication) and computes 128 queries of each
of its two batches. kf, powers, and attn@V are likewise computed only
for the pair. 4 distinct programs (one per pair shape), 2 cores each.

Per-core pipeline: host-pretransposed K/Q DMAs (plain full-bus loads),
kf = K@W_k on PE with the PSUM->SBUF copy fused with the clamp/scale,
powers t^2..t^9 chained and balanced over DVE/Pool/ACT(squares), Cq
built once on PE via beta-scaled-identity accumulation into PSUM with
ACT Copy(scale=w_v) writeback, ragged tails prefilled with -50 via a
ones@(-50/128) matmul, exp straight out of PSUM on ACT, attn@V + row
sums on PE, DVE reciprocal scaling, per-job output DMAs.
"""

import sys

if "/opt/trn_rl_repo" not in sys.path:
    sys.path.insert(0, "/opt/trn_rl_repo")

import numpy as np

import concourse.mybir as mybir
from concourse import tile, bacc
from concourse.bass_utils import run_bass_kernel_spmd

B, LQ, LK, DQ, DK, DV, H = 8, 256, 1024, 512, 512, 512, 128
N_CORES = 8
NEG = -50.0
NDC = DQ // 128   # contraction chunks
QPJ = 128         # queries per job (sub-batch) per core
NJ = 2            # jobs (batches) per core
NQ = NJ * QPJ     # query columns per core

A_CL = 3.4   # clamp for both qf and kf
C_SC = 1.9   # key-side power scaling: t = clamp(kf)/C_SC
M_V = 9      # key-side max power
DU = 10      # query-side Chebyshev degree

# Cq_m(u) = sum_i beta * T_i(clamp(u)/A_CL); ridge LS on the empirical
# qf/kf product distribution, pruned to 40 atoms (backward elimination,
# f16-validated end to end: rel err ~6.4e-3).
_CQ = {
    0: [(1, 1.22442424), (3, -0.30802553), (5, 0.11898142), (7, -0.03971164), (9, 0.02741055)],
    1: [(0, 0.39050645), (2, -0.59299057), (4, 0.42897732), (6, -0.24851432), (8, 0.08861760), (10, -0.09844897)],
    2: [(1, -0.24817115), (3, 0.45473564), (5, -0.47482833), (7, 0.25702238), (9, -0.29646627)],
    3: [(2, 0.09747580), (4, -0.40293099), (6, 0.53873099), (8, -0.20701768), (10, 0.44838648)],
    4: [(3, -0.09200685), (5, 0.33370477), (7, -0.29974551), (9, 0.49449164)],
    5: [(2, 0.00405760), (4, 0.07707166), (6, -0.38553593), (8, 0.11471073), (10, -0.55079067)],
    6: [(5, -0.06193477), (7, 0.13230420), (9, -0.25791133)],
    7: [(6, 0.11048150), (8, -0.01758446), (10, 0.24698473)],
    8: [(7, -0.02014013), (9, 0.04127641)],
    9: [(6, -0.01076934), (10, -0.03625577)],
}

_F16 = mybir.dt.float16
_F32 = mybir.dt.float32

_cached = {}


class _Bal:
    """Greedy engine balancer: track projected busy-ns for DVE/Pool/ACT."""

    def __init__(self):
        self.busy = {"dve": 0.0, "pool": 0.0, "act": 0.0}

    def add(self, eng, ns):
        self.busy[eng] += ns

    def pick(self, opts):
        best = min(opts, key=lambda o: self.busy[o[0]] + o[1])
        self.busy[best[0]] += best[1]
        return best[0]


def _ts4(n):   # DVE tensor_scalar f16 (4x)
    return (n / 4.0 + 58.0) / 0.96


def _tt2(n):   # DVE tensor_tensor / scalar_tensor_tensor f16 (2x)
    return (n / 2.0 + 58.0) / 0.96


def _cp1p(n):  # DVE f32-src PSUM->SBUF op (1x)
    return (n + 120.0) / 0.96


def _pool(n):  # Pool elementwise op
    return (n / 1.2) * 1.05 + 80.0


def _acts(n):  # ACT op, SBUF src
    return (n + 222.0) / 1.2 + 32.0


def _actp(n):  # ACT op, PSUM src
    return (n + 172.0) / 1.2 + 32.0


def _build(lens):
    """One core's program: NJ sub-batches with QPJ queries each."""
    nc = bacc.Bacc("TRN2", target_bir_lowering=False, debug=False)
    AL = mybir.AluOpType
    AF = mybir.ActivationFunctionType

    lens = [int(l) for l in lens]
    extents = [max(128, ((l + 127) // 128) * 128) for l in lens]
    nkcs = [e // 128 for e in extents]
    offs = np.concatenate([[0], np.cumsum(extents)]).astype(int)
    total_k = int(sum(extents))

    Qp = nc.declare_dram_parameter("Q", [128, NQ * DQ // 128], _F16, isOutput=False)
    Kp = nc.declare_dram_parameter("K", [128, total_k * DK // 128], _F16, isOutput=False)
    Vp = nc.declare_dram_parameter("V", [total_k, DV], _F16, isOutput=False)
    Wqp = nc.declare_dram_parameter("Wq", [128, DQ], _F16, isOutput=False)
    Wkp = nc.declare_dram_parameter("Wk", [128, DK], _F16, isOutput=False)
    outp = nc.declare_dram_parameter("out", [NJ, QPJ, DV], _F32, isOutput=True)
    wvp = nc.declare_dram_parameter("wv", [H, 1], _F32, isOutput=False)
    idp = nc.declare_dram_parameter("ident", [128, 128], _F16, isOutput=False)

    bal = _Bal()

    with tile.TileContext(nc) as tc:
        with (
            tc.tile_pool(name="const", bufs=1) as const,
            tc.tile_pool(name="cqt", bufs=4) as cqt,
            tc.tile_pool(name="kv", bufs=2) as kv,
            tc.tile_pool(name="pw", bufs=2) as pwp,
            tc.tile_pool(name="epool", bufs=2) as epool,
            tc.tile_pool(name="opool", bufs=2) as opool,
            tc.tile_pool(name="ps_s", bufs=1, space="PSUM") as ps_s,
            tc.tile_pool(name="ps_kf", bufs=2, space="PSUM") as ps_kf,
            tc.tile_pool(name="ps_tail", bufs=2, space="PSUM") as ps_tail,
        ):
            # ---- constants / weights -------------------------------------
            wq_sb = const.tile([128, NDC, H], _F16)
            nc.sync.dma_start(out=wq_sb, in_=Wqp[:, :].rearrange("p (c h) -> p c h", c=NDC))
            wk_sb = const.tile([128, NDC, H], _F16)
            nc.sync.dma_start(out=wk_sb, in_=Wkp[:, :].rearrange("p (c h) -> p c h", c=NDC))
            qT = const.tile([128, NDC, NQ], _F16)
            nc.sync.dma_start(out=qT, in_=Qp[:, :].rearrange("p (c q) -> p c q", c=NDC))
            wv_sb = const.tile([H, 1], _F32)
            nc.sync.dma_start(out=wv_sb, in_=wvp[:, :])
            ones = const.tile([128, NQ], _F16)
            nc.gpsimd.memset(ones, 1.0)
            negq = const.tile([128, QPJ], _F16)
            nc.gpsimd.memset(negq, NEG / 128.0)
            onecol = const.tile([128, 1], _F16)
            nc.gpsimd.memset(onecol, 1.0)
            ident = const.tile([128, 128], _F16)
            nc.sync.dma_start(out=ident, in_=idp[:, :])
            # warm the ACT table (exp/square/copy set) during initial DMAs
            warm = const.tile([128, 1], _F16)
            nc.scalar.activation(out=warm, in_=onecol, func=AF.Square, bias=0.0, scale=1.0)

            kts = {}
            vbs = {}

            def kdma(j):
                ext, nkc = extents[j], nkcs[j]
                o0 = int(offs[j])
                kT_b = kv.tile([128, NDC, ext], _F16, tag="kT")
                nc.sync.dma_start(
                    out=kT_b,
                    in_=Kp[:, :].rearrange("p (c k) -> p c k", c=NDC)[:, :, o0 : o0 + ext],
                )
                v_b = kv.tile([128, nkc, DV], _F16, tag="v")
                nc.sync.dma_start(
                    out=v_b, in_=Vp[o0 : o0 + ext, :].rearrange("(c p) d -> p c d", p=128)
                )
                kts[j] = kT_b
                vbs[j] = v_b

            kdma(0)
            kdma(1)

            qf_ps = ps_tail.tile([128, NQ], _F32, tag="tail")
            for dc in range(NDC):
                nc.tensor.matmul(
                    out=qf_ps, lhsT=wq_sb[:, dc, :], rhs=qT[:, dc, :],
                    start=(dc == 0), stop=(dc == NDC - 1),
                )
            ucl = const.tile([128, NQ], _F16, name="ucl")
            nc.vector.tensor_scalar(
                out=ucl, in0=qf_ps, scalar1=1.0 / A_CL, scalar2=1.0,
                op0=AL.mult, op1=AL.min,
            )
            bal.add("dve", _cp1p(NQ))
            nc.vector.tensor_scalar(out=ucl, in0=ucl, scalar1=-1.0, scalar2=None, op0=AL.max)
            bal.add("dve", _ts4(NQ))

            # Chebyshev T_0..T_DU by doubling: T_2i = 2*T_i^2-1 (ACT square
            # + ts), T_2i+1 = 2*T_i*T_{i+1} - T_1 (tt + stt).
            T = [ones, ucl] + [None] * (DU - 1)

            def emit_T(i):
                if T[i] is not None:
                    return T[i]
                a = i // 2
                ti = const.tile([128, NQ], _F16, name=f"T{i}")
                if i % 2 == 0:
                    src = emit_T(a)
                    sq = cqt.tile([128, NQ], _F16, tag="ct", name=f"sq{i}", bufs=3)
                    nc.scalar.activation(out=sq, in_=src, func=AF.Square, bias=0.0, scale=1.0)
                    bal.add("act", _acts(NQ))
                    eng = bal.pick([("dve", _ts4(NQ)), ("pool", _pool(NQ))])
                    e = nc.vector if eng == "dve" else nc.gpsimd
                    e.tensor_scalar(out=ti, in0=sq, scalar1=2.0, scalar2=-1.0,
                                    op0=AL.mult, op1=AL.add)
                else:
                    s0, s1 = emit_T(a), emit_T(a + 1)
                    tmp = cqt.tile([128, NQ], _F16, tag="ct", name=f"tm{i}", bufs=3)
                    eng = bal.pick([("dve", _tt2(NQ)), ("pool", _pool(NQ))])
                    e = nc.vector if eng == "dve" else nc.gpsimd
                    e.tensor_tensor(out=tmp, in0=s0, in1=s1, op=AL.mult)
                    eng = bal.pick([("dve", _tt2(NQ)), ("pool", 2 * _pool(NQ))])
                    if eng == "dve":
                        nc.vector.scalar_tensor_tensor(
                            out=ti, in0=tmp, scalar=2.0, in1=ucl,
                            op0=AL.mult, op1=AL.subtract,
                        )
                    else:
                        p2t = cqt.tile([128, NQ], _F16, tag="ct", name=f"p2{i}", bufs=3)
                        nc.gpsimd.tensor_scalar(out=p2t, in0=tmp, scalar1=2.0,
                                                scalar2=None, op0=AL.mult)
                        nc.gpsimd.tensor_tensor(out=ti, in0=p2t, in1=ucl, op=AL.subtract)
                T[i] = ti
                return ti

            for i in range(2, DU + 1):
                emit_T(i)

            # Cq_m on PE: beta-scaled identity accumulation in PSUM; the
            # PSUM->SBUF copy is an ACT Copy with per-partition scale w_v.
            cq = {}

            def cq_build(ms):
                pair_ps = ps_kf.tile([128, len(ms), NQ], _F32, tag="kf")
                for j, m in enumerate(ms):
                    items = _CQ[m]
                    for a, (i, b_) in enumerate(items):
                        sid = cqt.tile([128, 128], _F16, tag="sid", name=f"s{m}_{i}", bufs=6)
                        eng = bal.pick([("dve", _ts4(128)), ("pool", _pool(128))])
                        e = nc.vector if eng == "dve" else nc.gpsimd
                        e.tensor_scalar(out=sid, in0=ident, scalar1=float(b_),
                                        scalar2=None, op0=AL.mult)
                        nc.tensor.matmul(
                            out=pair_ps[:, j, :], lhsT=sid, rhs=T[i],
                            start=(a == 0), stop=(a == len(items) - 1),
                        )
                    cqm = const.tile([128, NQ], _F16, name=f"cq{m}")
                    nc.scalar.activation(out=cqm, in_=pair_ps[:, j, :], func=AF.Copy,
                                         bias=0.0, scale=wv_sb[:, 0:1])
                    bal.add("act", _actp(NQ))
                    cq[m] = cqm

            # ---- per-job K path: kf, clamp, powers -----------------------
            pows = {}

            def kf_path(j):
                ext, nkc, ln = extents[j], nkcs[j], lens[j]
                kT_b = kts.pop(j)
                t_b = pwp.tile([128, LK], _F16, tag="pw1")
                for c0 in range(0, ln, 512):
                    cn = min(512, ln - c0)
                    kf_ps = ps_kf.tile([128, 512], _F32, tag="kf")
                    for dc in range(NDC):
                        nc.tensor.matmul(
                            out=kf_ps[:, 0:cn],
                            lhsT=wk_sb[:, dc, :],
                            rhs=kT_b[:, dc, c0 : c0 + cn],
                            start=(dc == 0),
                            stop=(dc == NDC - 1),
                        )
                    nc.vector.tensor_scalar(
                        out=t_b[:, c0 : c0 + cn], in0=kf_ps[:, 0:cn],
                        scalar1=1.0 / C_SC, scalar2=A_CL / C_SC,
                        op0=AL.mult, op1=AL.min,
                    )
                    bal.add("dve", _cp1p(cn))
                eng = bal.pick([("dve", _ts4(ln)), ("pool", _pool(ln))])
                e = nc.vector if eng == "dve" else nc.gpsimd
                e.tensor_scalar(out=t_b[:, 0:ln], in0=t_b[:, 0:ln],
                                scalar1=-A_CL / C_SC, scalar2=None, op0=AL.max)
                P = {1: t_b}
                for m in range(2, M_V + 1):
                    pm = pwp.tile([128, LK], _F16, tag=f"pw{m}")
                    a, c = m // 2, m - m // 2
                    opts = [("dve", _tt2(ln)), ("pool", _pool(ln))]
                    if a == c:
                        opts.append(("act", _acts(ln)))
                    eng = bal.pick(opts)
                    if eng == "act":
                        nc.scalar.activation(out=pm[:, 0:ln], in_=P[a][:, 0:ln],
                                             func=AF.Square, bias=0.0, scale=1.0)
                    else:
                        e = nc.vector if eng == "dve" else nc.gpsimd
                        e.tensor_tensor(out=pm[:, 0:ln], in0=P[a][:, 0:ln],
                                        in1=P[c][:, 0:ln], op=AL.mult)
                    P[m] = pm
                pows[j] = P

            def job_pipeline(j, halves):
                """scores -> exp -> attn@V for job j, pipelined over chunk
                groups: exp/attnV of group g overlap scores of group g+1
                (o_ps/rs accumulation groups live in different PSUM banks
                than s_ps, so they may stay open across score matmuls)."""
                nkc, ln = nkcs[j], lens[j]
                v_b = vbs.pop(j)
                P = pows.pop(j)
                s_ps = ps_s.tile([128, nkc, QPJ], _F32, tag="s")
                rl = ln - 128 * (nkc - 1)
                if rl < 128:
                    base = 96 if rl >= 96 else (64 if rl >= 64 else 0)
                    nc.tensor.matmul(
                        out=s_ps[base:128, nkc - 1, :], lhsT=ones[:, 0 : 128 - base],
                        rhs=negq, start=True, stop=True,
                        skip_group_check=True, tile_position=(0, base),
                    )
                e_b = epool.tile([128, nkc, QPJ], _F16, tag="e")
                o_ps = ps_tail.tile([QPJ, DV], _F32, tag="tail")
                rs_ps = ps_kf.tile([QPJ, 1], _F32, tag="kf")
                bounds = []
                g0 = 0
                for g in range(halves):
                    g1 = nkc * (g + 1) // halves
                    if g1 > g0:
                        bounds.append((g0, g1))
                        g0 = g1
                for gi, (k0, k1) in enumerate(bounds):
                    for kc in range(k0, k1):
                        r = min(128, ln - kc * 128)
                        for m in range(M_V + 1):
                            lhsT = (ones[:, 0:r] if m == 0
                                    else P[m][:, kc * 128 : kc * 128 + r])
                            nc.tensor.matmul(
                                out=s_ps[0:r, kc, :],
                                lhsT=lhsT,
                                rhs=cq[m][:, j * QPJ : (j + 1) * QPJ],
                                start=(m == 0),
                                stop=(m == M_V),
                            )
                    nc.scalar.activation(out=e_b[:, k0:k1, :], in_=s_ps[:, k0:k1, :],
                                         func=AF.Exp, bias=0.0, scale=1.0)
                    bal.add("act", _actp((k1 - k0) * QPJ))
                    for kc in range(k0, k1):
                        nc.tensor.matmul(
                            out=o_ps, lhsT=e_b[:, kc, :], rhs=v_b[:, kc, :],
                            start=(kc == 0), stop=(kc == nkc - 1),
                        )
                        nc.tensor.matmul(
                            out=rs_ps, lhsT=e_b[:, kc, :], rhs=onecol,
                            start=(kc == 0), stop=(kc == nkc - 1),
                        )
                rinv = opool.tile([QPJ, 1], _F32, tag="ri")
                nc.vector.reciprocal(rinv, rs_ps)
                bal.add("dve", 130.0)
                osb = opool.tile([QPJ, DV], _F32, tag="o")
                eng = bal.pick([("dve", _cp1p(DV)), ("act", _actp(DV))])
                if eng == "act":
                    nc.scalar.activation(out=osb, in_=o_ps,
                                         func=AF.Copy, bias=0.0, scale=rinv[:, 0:1])
                else:
                    nc.vector.tensor_scalar(
                        out=osb, in0=o_ps, scalar1=rinv[:, 0:1],
                        scalar2=None, op0=AL.mult,
                    )
                nc.sync.dma_start(out=outp[j, :, :], in_=osb)

            # ---- two-job schedule ---------------------------------------
            kf_path(0)
            cq_build([0, 1])
            cq_build([2, 3])
            kf_path(1)
            cq_build([4, 5])
            cq_build([6, 7])
            cq_build([8, 9])
            job_pipeline(0, halves=max(1, nkcs[0] // 2))
            job_pipeline(1, halves=1)

    nc.finalize()
    return nc


def _pairing(lens):
    """Pair largest with smallest by extent; returns list of (ja, jb)."""
    order = sorted(range(B), key=lambda b: (-int(lens[b]), b))
    return [(order[i], order[B - 1 - i]) for i in range(B // 2)]


def _get_nc(lens_pair):
    key = tuple(int(l) for l in lens_pair)
    if key not in _cached:
        _cached[key] = _build(key)
    return _cached[key]


def _prep_T(x):
    """[rows, 512] -> [128, 4*rows] host pre-transpose (chunk-major)."""
    r = x.shape[0]
    return np.ascontiguousarray(x.reshape(r, NDC, 128).transpose(2, 1, 0).reshape(128, -1))


def kernel(Q, K, V, valid_lengths, W_q, W_k, w_v):
    Q = np.asarray(Q, dtype=np.float32)
    K = np.asarray(K, dtype=np.float32)
    V = np.asarray(V, dtype=np.float32)
    vl = np.asarray(valid_lengths).astype(np.int64).reshape(B)
    W_q = np.asarray(W_q, dtype=np.float32)
    W_k = np.asarray(W_k, dtype=np.float32)
    w_v = np.asarray(w_v, dtype=np.float32)

    lens = np.clip(vl, 1, LK)
    extents = np.clip(np.ceil(lens / 128.0).astype(int) * 128, 128, LK)
    pairs = _pairing(lens)

    f16 = np.float16
    # weights: [512, 128] -> [128, (c h)] with row p holding W[c*128+p, h]
    Wqb = np.ascontiguousarray(
        W_q.reshape(NDC, 128, H).transpose(1, 0, 2).reshape(128, DQ)
    ).astype(f16)
    Wkb = np.ascontiguousarray(
        W_k.reshape(NDC, 128, H).transpose(1, 0, 2).reshape(128, DK)
    ).astype(f16)
    wvb = w_v.reshape(H, 1).astype(np.float32)
    Qb = Q.astype(f16)
    eye = np.eye(128, dtype=f16)

    out = np.empty((B, LQ, DV), dtype=np.float32)
    for p, (ja, jb) in enumerate(pairs):
        nc = _get_nc((lens[ja], lens[jb]))
        KT = np.concatenate(
            [
                _prep_T(K[j, : extents[j], :].astype(np.float32)).reshape(128, NDC, -1)
                for j in (ja, jb)
            ],
            axis=2,
        ).reshape(128, -1).astype(f16)
        Vc = np.concatenate(
            [V[j, : extents[j], :] for j in (ja, jb)], axis=0
        ).astype(f16)
        in_maps = []
        for h in range(2):
            Qcore = np.concatenate(
                [Qb[j, h * QPJ : (h + 1) * QPJ, :] for j in (ja, jb)], axis=0
            )
            in_maps.append(
                {"Q": _prep_T(Qcore.astype(np.float32)).astype(f16), "K": KT,
                 "V": Vc, "Wq": Wqb, "Wk": Wkb, "wv": wvb, "ident": eye}
            )
        res = run_bass_kernel_spmd(nc, in_maps, core_ids=[2 * p, 2 * p + 1])
        for h in range(2):
            oc = res.results[h]["out"]  # (NJ, QPJ, DV)
            out[ja, h * QPJ : (h + 1) * QPJ, :] = oc[0]
            out[jb, h * QPJ : (h + 1) * QPJ, :] = oc[1]
    return out


# revision 39
# speedup vs baseline: 1.0385x; 1.0385x over previous
"""Additive (Bahdanau) attention on 8 TRN2 NeuronCores.

Problem: B=8, LQ=256, LK=1024, DQ=DK=DV=512, H=128.
  q = Q @ W_q; k = K @ W_k
  scores[b,q,k] = sum_h w_v[h] * tanh(qf[b,q,h] + kf[b,k,h])
  out = softmax_k(mask(scores)) @ V

Factorized-score formulation (replaces the O(LQ*LK*H) elementwise tanh):
  tanh(u+v) ~= sum_{m=0..9} Cq_m(u) * t(v)^m,   t = clamp(v,+-3.4)/1.9
with Cq_m(u) = sum_i beta[i,m] T_i(clamp(u,+-3.4)/3.4) a 40-atom pruned
Chebyshev fit (offline ridge LS on the empirical qf/kf distribution;
end-to-end rel err ~6.4e-3 including all f16 effects). Scores become 10
accumulating PE matmuls per 128-key chunk:
  scores[k,q] = sum_m matmul(lhsT = t^m [h,k], rhs = (w_v o Cq_m) [h,q])

Sharding: batches are paired (largest with smallest valid length) onto
core pairs. Each core loads ONLY its pair's K/V (about 2.6MB instead of
the full 9.4MB - no 8x DMA replication) and computes 128 queries of each
of its two batches. kf, powers, and attn@V are likewise computed only
for the pair. 4 distinct programs (one per pair shape), 2 cores each.

Per-core pipeline: host-pretransposed K/Q DMAs (plain full-bus loads),
kf = K@W_k on PE with the PSUM->SBUF copy fused with the clamp/scale,
powers t^2..t^9 chained and balanced over DVE/Pool/ACT(squares), Cq
built once on PE via beta-scaled-identity accumulation into PSUM with
ACT Copy(scale=w_v) writeback, ragged tails prefilled with -50 via a
ones@(-50/128) matmul, exp straight out of PSUM on ACT, attn@V + row
sums on PE, DVE reciprocal scaling, per-job output DMAs.
"""

import sys

if "/opt/trn_rl_repo" not in sys.path:
    sys.path.insert(0, "/opt/trn_rl_repo")

import numpy as np

import concourse.mybir as mybir
from concourse import tile, bacc
from concourse.bass_utils import run_bass_kernel_spmd

B, LQ, LK, DQ, DK, DV, H = 8, 256, 1024, 512, 512, 512, 128
N_CORES = 8
NEG = -50.0
NDC = DQ // 128   # contraction chunks
QPJ = 128         # queries per job (sub-batch) per core
NJ = 2            # jobs (batches) per core
NQ = NJ * QPJ     # query columns per core

A_CL = 3.4   # clamp for both qf and kf
C_SC = 1.9   # key-side power scaling: t = clamp(kf)/C_SC
M_V = 9      # key-side max power
DU = 10      # query-side Chebyshev degree

# Cq_m(u) = sum_i beta * T_i(clamp(u)/A_CL); ridge LS on the empirical
# qf/kf product distribution, pruned to 40 atoms (backward elimination,
# f16-validated end to end: rel err ~6.4e-3).
_CQ = {
    0: [(1, 1.22442424), (3, -0.30802553), (5, 0.11898142), (7, -0.03971164), (9, 0.02741055)],
    1: [(0, 0.39050645), (2, -0.59299057), (4, 0.42897732), (6, -0.24851432), (8, 0.08861760), (10, -0.09844897)],
    2: [(1, -0.24817115), (3, 0.45473564), (5, -0.47482833), (7, 0.25702238), (9, -0.29646627)],
    3: [(2, 0.09747580), (4, -0.40293099), (6, 0.53873099), (8, -0.20701768), (10, 0.44838648)],
    4: [(3, -0.09200685), (5, 0.33370477), (7, -0.29974551), (9, 0.49449164)],
    5: [(2, 0.00405760), (4, 0.07707166), (6, -0.38553593), (8, 0.11471073), (10, -0.55079067)],
    6: [(5, -0.06193477), (7, 0.13230420), (9, -0.25791133)],
    7: [(6, 0.11048150), (8, -0.01758446), (10, 0.24698473)],
    8: [(7, -0.02014013), (9, 0.04127641)],
    9: [(6, -0.01076934), (10, -0.03625577)],
}

_F16 = mybir.dt.float16
_F32 = mybir.dt.float32

_cached = {}


class _Bal:
    """Greedy engine balancer: track projected busy-ns for DVE/Pool/ACT."""

    def __init__(self):
        self.busy = {"dve": 0.0, "pool": 0.0, "act": 0.0}

    def add(self, eng, ns):
        self.busy[eng] += ns

    def pick(self, opts):
        best = min(opts, key=lambda o: self.busy[o[0]] + o[1])
        self.busy[best[0]] += best[1]
        return best[0]


def _ts4(n):   # DVE tensor_scalar f16 (4x)
    return (n / 4.0 + 58.0) / 0.96


def _tt2(n):   # DVE tensor_tensor / scalar_tensor_tensor f16 (2x)
    return (n / 2.0 + 58.0) / 0.96


def _cp1p(n):  # DVE f32-src PSUM->SBUF op (1x)
    return (n + 120.0) / 0.96


def _pool(n):  # Pool elementwise op
    return (n / 1.2) * 1.05 + 80.0


def _acts(n):  # ACT op, SBUF src
    return (n + 222.0) / 1.2 + 32.0


def _actp(n):  # ACT op, PSUM src
    return (n + 172.0) / 1.2 + 32.0


def _build(lens):
    """One core's program: NJ sub-batches with QPJ queries each."""
    nc = bacc.Bacc("TRN2", target_bir_lowering=False, debug=False)
    AL = mybir.AluOpType
    AF = mybir.ActivationFunctionType

    lens = [int(l) for l in lens]
    extents = [max(128, ((l + 127) // 128) * 128) for l in lens]
    nkcs = [e // 128 for e in extents]
    offs = np.concatenate([[0], np.cumsum(extents)]).astype(int)
    total_k = int(sum(extents))

    Qp = nc.declare_dram_parameter("Q", [128, NQ * DQ // 128], _F16, isOutput=False)
    Kp = nc.declare_dram_parameter("K", [128, total_k * DK // 128], _F16, isOutput=False)
    Vp = nc.declare_dram_parameter("V", [total_k, DV], _F16, isOutput=False)
    Wqp = nc.declare_dram_parameter("Wq", [128, DQ], _F16, isOutput=False)
    Wkp = nc.declare_dram_parameter("Wk", [128, DK], _F16, isOutput=False)
    outp = nc.declare_dram_parameter("out", [NJ, QPJ, DV], _F32, isOutput=True)
    wvp = nc.declare_dram_parameter("wv", [H, 1], _F32, isOutput=False)
    idp = nc.declare_dram_parameter("ident", [128, 128], _F16, isOutput=False)

    bal = _Bal()

    with tile.TileContext(nc) as tc:
        with (
            tc.tile_pool(name="const", bufs=1) as const,
            tc.tile_pool(name="cqt", bufs=4) as cqt,
            tc.tile_pool(name="kv", bufs=2) as kv,
            tc.tile_pool(name="pw", bufs=2) as pwp,
            tc.tile_pool(name="epool", bufs=2) as epool,
            tc.tile_pool(name="opool", bufs=2) as opool,
            tc.tile_pool(name="ps_s", bufs=1, space="PSUM") as ps_s,
            tc.tile_pool(name="ps_kf", bufs=2, space="PSUM") as ps_kf,
            tc.tile_pool(name="ps_tail", bufs=2, space="PSUM") as ps_tail,
        ):
            # ---- constants / weights -------------------------------------
            wq_sb = const.tile([128, NDC, H], _F16)
            nc.sync.dma_start(out=wq_sb, in_=Wqp[:, :].rearrange("p (c h) -> p c h", c=NDC))
            wk_sb = const.tile([128, NDC, H], _F16)
            nc.sync.dma_start(out=wk_sb, in_=Wkp[:, :].rearrange("p (c h) -> p c h", c=NDC))
            qT = const.tile([128, NDC, NQ], _F16)
            nc.sync.dma_start(out=qT, in_=Qp[:, :].rearrange("p (c q) -> p c q", c=NDC))
            wv_sb = const.tile([H, 1], _F32)
            nc.sync.dma_start(out=wv_sb, in_=wvp[:, :])
            ones = const.tile([128, NQ], _F16)
            nc.gpsimd.memset(ones, 1.0)
            negq = const.tile([128, QPJ], _F16)
            nc.gpsimd.memset(negq, NEG / 128.0)
            onecol = const.tile([128, 1], _F16)
            nc.gpsimd.memset(onecol, 1.0)
            ident = const.tile([128, 128], _F16)
            nc.sync.dma_start(out=ident, in_=idp[:, :])
            # warm the ACT table (exp/square/copy set) during initial DMAs
            warm = const.tile([128, 1], _F16)
            nc.scalar.activation(out=warm, in_=onecol, func=AF.Square, bias=0.0, scale=1.0)

            kts = {}
            vbs = {}

            def kdma(j):
                ext, nkc = extents[j], nkcs[j]
                o0 = int(offs[j])
                kT_b = kv.tile([128, NDC, ext], _F16, tag="kT")
                nc.sync.dma_start(
                    out=kT_b,
                    in_=Kp[:, :].rearrange("p (c k) -> p c k", c=NDC)[:, :, o0 : o0 + ext],
                )
                v_b = kv.tile([128, nkc, DV], _F16, tag="v")
                nc.sync.dma_start(
                    out=v_b, in_=Vp[o0 : o0 + ext, :].rearrange("(c p) d -> p c d", p=128)
                )
                kts[j] = kT_b
                vbs[j] = v_b

            kdma(0)
            kdma(1)

            qf_ps = ps_tail.tile([128, NQ], _F32, tag="tail")
            for dc in range(NDC):
                nc.tensor.matmul(
                    out=qf_ps, lhsT=wq_sb[:, dc, :], rhs=qT[:, dc, :],
                    start=(dc == 0), stop=(dc == NDC - 1),
                )
            ucl = const.tile([128, NQ], _F16, name="ucl")
            nc.vector.tensor_scalar(
                out=ucl, in0=qf_ps, scalar1=1.0 / A_CL, scalar2=1.0,
                op0=AL.mult, op1=AL.min,
            )
            bal.add("dve", _cp1p(NQ))
            nc.vector.tensor_scalar(out=ucl, in0=ucl, scalar1=-1.0, scalar2=None, op0=AL.max)
            bal.add("dve", _ts4(NQ))

            # Chebyshev T_0..T_DU by doubling: T_2i = 2*T_i^2-1 (ACT square
            # + ts), T_2i+1 = 2*T_i*T_{i+1} - T_1 (tt + stt).
            T = [ones, ucl] + [None] * (DU - 1)

            def emit_T(i):
                if T[i] is not None:
                    return T[i]
                a = i // 2
                ti = const.tile([128, NQ], _F16, name=f"T{i}")
                if i % 2 == 0:
                    src = emit_T(a)
                    sq = cqt.tile([128, NQ], _F16, tag="ct", name=f"sq{i}", bufs=3)
                    nc.scalar.activation(out=sq, in_=src, func=AF.Square, bias=0.0, scale=1.0)
                    bal.add("act", _acts(NQ))
                    eng = bal.pick([("dve", _ts4(NQ)), ("pool", _pool(NQ))])
                    e = nc.vector if eng == "dve" else nc.gpsimd
                    e.tensor_scalar(out=ti, in0=sq, scalar1=2.0, scalar2=-1.0,
                                    op0=AL.mult, op1=AL.add)
                else:
                    s0, s1 = emit_T(a), emit_T(a + 1)
                    tmp = cqt.tile([128, NQ], _F16, tag="ct", name=f"tm{i}", bufs=3)
                    eng = bal.pick([("dve", _tt2(NQ)), ("pool", _pool(NQ))])
                    e = nc.vector if eng == "dve" else nc.gpsimd
                    e.tensor_tensor(out=tmp, in0=s0, in1=s1, op=AL.mult)
                    eng = bal.pick([("dve", _tt2(NQ)), ("pool", 2 * _pool(NQ))])
                    if eng == "dve":
                        nc.vector.scalar_tensor_tensor(
                            out=ti, in0=tmp, scalar=2.0, in1=ucl,
                            op0=AL.mult, op1=AL.subtract,
                        )
                    else:
                        p2t = cqt.tile([128, NQ], _F16, tag="ct", name=f"p2{i}", bufs=3)
                        nc.gpsimd.tensor_scalar(out=p2t, in0=tmp, scalar1=2.0,
                                                scalar2=None, op0=AL.mult)
                        nc.gpsimd.tensor_tensor(out=ti, in0=p2t, in1=ucl, op=AL.subtract)
                T[i] = ti
                return ti

            for i in range(2, DU + 1):
                emit_T(i)

            # Cq_m on PE: beta-scaled identity accumulation in PSUM; the
            # PSUM->SBUF copy is an ACT Copy with per-partition scale w_v.
            cq = {}

            def cq_build(ms):
                pair_ps = ps_kf.tile([128, len(ms), NQ], _F32, tag="kf")
                for j, m in enumerate(ms):
                    items = _CQ[m]
                    for a, (i, b_) in enumerate(items):
                        sid = cqt.tile([128, 128], _F16, tag="sid", name=f"s{m}_{i}", bufs=6)
                        eng = bal.pick([("dve", _ts4(128)), ("pool", _pool(128))])
                        e = nc.vector if eng == "dve" else nc.gpsimd
                        e.tensor_scalar(out=sid, in0=ident, scalar1=float(b_),
                                        scalar2=None, op0=AL.mult)
                        nc.tensor.matmul(
                            out=pair_ps[:, j, :], lhsT=sid, rhs=T[i],
                            start=(a == 0), stop=(a == len(items) - 1),
                        )
                    cqm = const.tile([128, NQ], _F16, name=f"cq{m}")
                    nc.scalar.activation(out=cqm, in_=pair_ps[:, j, :], func=AF.Copy,
                                         bias=0.0, scale=wv_sb[:, 0:1])
                    bal.add("act", _actp(NQ))
                    cq[m] = cqm

            # ---- per-job K path: kf, clamp, powers -----------------------
            pows = {}

            def kf_path(j):
                ext, nkc, ln = extents[j], nkcs[j], lens[j]
                kT_b = kts.pop(j)
                t_b = pwp.tile([128, LK], _F16, tag="pw1")
                for c0 in range(0, ln, 512):
                    cn = min(512, ln - c0)
                    kf_ps = ps_kf.tile([128, 512], _F32, tag="kf")
                    for dc in range(NDC):
                        nc.tensor.matmul(
                            out=kf_ps[:, 0:cn],
                            lhsT=wk_sb[:, dc, :],
                            rhs=kT_b[:, dc, c0 : c0 + cn],
                            start=(dc == 0),
                            stop=(dc == NDC - 1),
                        )
                    nc.vector.tensor_scalar(
                        out=t_b[:, c0 : c0 + cn], in0=kf_ps[:, 0:cn],
                        scalar1=1.0 / C_SC, scalar2=A_CL / C_SC,
                        op0=AL.mult, op1=AL.min,
                    )
                    bal.add("dve", _cp1p(cn))
                eng = bal.pick([("dve", _ts4(ln)), ("pool", _pool(ln))])
                e = nc.vector if eng == "dve" else nc.gpsimd
                e.tensor_scalar(out=t_b[:, 0:ln], in0=t_b[:, 0:ln],
                                scalar1=-A_CL / C_SC, scalar2=None, op0=AL.max)
                P = {1: t_b}
                for m in range(2, M_V + 1):
                    pm = pwp.tile([128, LK], _F16, tag=f"pw{m}")
                    a, c = m // 2, m - m // 2
                    opts = [("dve", _tt2(ln)), ("pool", _pool(ln))]
                    if a == c:
                        opts.append(("act", _acts(ln)))
                    eng = bal.pick(opts)
                    if eng == "act":
                        nc.scalar.activation(out=pm[:, 0:ln], in_=P[a][:, 0:ln],
                                             func=AF.Square, bias=0.0, scale=1.0)
                    else:
                        e = nc.vector if eng == "dve" else nc.gpsimd
                        e.tensor_tensor(out=pm[:, 0:ln], in0=P[a][:, 0:ln],
                                        in1=P[c][:, 0:ln], op=AL.mult)
                    P[m] = pm
                pows[j] = P

            def job_pipeline(j, halves):
                """scores -> exp -> attn@V for job j, pipelined over chunk
                groups: exp/attnV of group g overlap scores of group g+1
                (o_ps/rs accumulation groups live in different PSUM banks
                than s_ps, so they may stay open across score matmuls)."""
                nkc, ln = nkcs[j], lens[j]
                v_b = vbs.pop(j)
                P = pows.pop(j)
                s_ps = ps_s.tile([128, nkc, QPJ], _F32, tag="s")
                rl = ln - 128 * (nkc - 1)
                if rl < 128:
                    base = 96 if rl >= 96 else (64 if rl >= 64 else 0)
                    nc.tensor.matmul(
                        out=s_ps[base:128, nkc - 1, :], lhsT=ones[:, 0 : 128 - base],
                        rhs=negq, start=True, stop=True,
                        skip_group_check=True, tile_position=(0, base),
                    )
                e_b = epool.tile([128, nkc, QPJ], _F16, tag="e")
                o_ps = ps_tail.tile([QPJ, DV], _F32, tag="tail")
                rs_ps = ps_kf.tile([QPJ, 1], _F32, tag="kf")
                bounds = []
                g0 = 0
                for g in range(halves):
                    g1 = nkc * (g + 1) // halves
                    if g1 > g0:
                        bounds.append((g0, g1))
                        g0 = g1
                for gi, (k0, k1) in enumerate(bounds):
                    for kc in range(k0, k1):
                        r = min(128, ln - kc * 128)
                        for m in range(M_V + 1):
                            lhsT = (ones[:, 0:r] if m == 0
                                    else P[m][:, kc * 128 : kc * 128 + r])
                            nc.tensor.matmul(
                                out=s_ps[0:r, kc, :],
                                lhsT=lhsT,
                                rhs=cq[m][:, j * QPJ : (j + 1) * QPJ],
                                start=(m == 0),
                                stop=(m == M_V),
                            )
                    # exp of this chunk group runs on ACT while PE streams
                    # the next group's score matmuls
                    nc.scalar.activation(out=e_b[:, k0:k1, :], in_=s_ps[:, k0:k1, :],
                                         func=AF.Exp, bias=0.0, scale=1.0)
                    bal.add("act", _actp((k1 - k0) * QPJ))
                for kc in range(nkc):
                    nc.tensor.matmul(
                        out=o_ps, lhsT=e_b[:, kc, :], rhs=v_b[:, kc, :],
                        start=(kc == 0), stop=(kc == nkc - 1),
                    )
                    nc.tensor.matmul(
                        out=rs_ps, lhsT=e_b[:, kc, :], rhs=onecol,
                        start=(kc == 0), stop=(kc == nkc - 1),
                    )
                rinv = opool.tile([QPJ, 1], _F32, tag="ri")
                nc.vector.reciprocal(rinv, rs_ps)
                bal.add("dve", 130.0)
                osb = opool.tile([QPJ, DV], _F32, tag="o")
                eng = bal.pick([("dve", _cp1p(DV)), ("act", _actp(DV))])
                if eng == "act":
                    nc.scalar.activation(out=osb, in_=o_ps,
                                         func=AF.Copy, bias=0.0, scale=rinv[:, 0:1])
                else:
                    nc.vector.tensor_scalar(
                        out=osb, in0=o_ps, scalar1=rinv[:, 0:1],
                        scalar2=None, op0=AL.mult,
                    )
                nc.sync.dma_start(out=outp[j, :, :], in_=osb)

            # ---- two-job schedule ---------------------------------------
            kf_path(0)
            cq_build([0, 1])
            cq_build([2, 3])
            kf_path(1)
            cq_build([4, 5])
            cq_build([6, 7])
            cq_build([8, 9])
            job_pipeline(0, halves=max(1, nkcs[0] // 2))
            job_pipeline(1, halves=1)

    nc.finalize()
    return nc


def _pairing(lens):
    """Pair largest with smallest by extent; returns list of (ja, jb)."""
    order = sorted(range(B), key=lambda b: (-int(lens[b]), b))
    return [(order[i], order[B - 1 - i]) for i in range(B // 2)]


def _get_nc(lens_pair):
    key = tuple(int(l) for l in lens_pair)
    if key not in _cached:
        _cached[key] = _build(key)
    return _cached[key]


def _prep_T(x):
    """[rows, 512] -> [128, 4*rows] host pre-transpose (chunk-major)."""
    r = x.shape[0]
    return np.ascontiguousarray(x.reshape(r, NDC, 128).transpose(2, 1, 0).reshape(128, -1))


def kernel(Q, K, V, valid_lengths, W_q, W_k, w_v):
    Q = np.asarray(Q, dtype=np.float32)
    K = np.asarray(K, dtype=np.float32)
    V = np.asarray(V, dtype=np.float32)
    vl = np.asarray(valid_lengths).astype(np.int64).reshape(B)
    W_q = np.asarray(W_q, dtype=np.float32)
    W_k = np.asarray(W_k, dtype=np.float32)
    w_v = np.asarray(w_v, dtype=np.float32)

    lens = np.clip(vl, 1, LK)
    extents = np.clip(np.ceil(lens / 128.0).astype(int) * 128, 128, LK)
    pairs = _pairing(lens)

    f16 = np.float16
    # weights: [512, 128] -> [128, (c h)] with row p holding W[c*128+p, h]
    Wqb = np.ascontiguousarray(
        W_q.reshape(NDC, 128, H).transpose(1, 0, 2).reshape(128, DQ)
    ).astype(f16)
    Wkb = np.ascontiguousarray(
        W_k.reshape(NDC, 128, H).transpose(1, 0, 2).reshape(128, DK)
    ).astype(f16)
    wvb = w_v.reshape(H, 1).astype(np.float32)
    Qb = Q.astype(f16)
    eye = np.eye(128, dtype=f16)

    out = np.empty((B, LQ, DV), dtype=np.float32)
    for p, (ja, jb) in enumerate(pairs):
        nc = _get_nc((lens[ja], lens[jb]))
        KT = np.concatenate(
            [
                _prep_T(K[j, : extents[j], :].astype(np.float32)).reshape(128, NDC, -1)
                for j in (ja, jb)
            ],
            axis=2,
        ).reshape(128, -1).astype(f16)
        Vc = np.concatenate(
            [V[j, : extents[j], :] for j in (ja, jb)], axis=0
        ).astype(f16)
        in_maps = []
        for h in range(2):
            Qcore = np.concatenate(
                [Qb[j, h * QPJ : (h + 1) * QPJ, :] for j in (ja, jb)], axis=0
            )
            in_maps.append(
                {"Q": _prep_T(Qcore.astype(np.float32)).astype(f16), "K": KT,
                 "V": Vc, "Wq": Wqb, "Wk": Wkb, "wv": wvb, "ident": eye}
            )
        res = run_bass_kernel_spmd(nc, in_maps, core_ids=[2 * p, 2 * p + 1])
        for h in range(2):
            oc = res.results[h]["out"]  # (NJ, QPJ, DV)
            out[ja, h * QPJ : (h + 1) * QPJ, :] = oc[0]
            out[jb, h * QPJ : (h + 1) * QPJ, :] = oc[1]
    return out
